# revision 18
# baseline (speedup 1.0000x reference)
"""EntMaxSelectLayer distributed Trainium2 kernel (v2).

Computes out = x @ entmax15(weight, axis=-1) with
  x [512, 8192] f32, weight [8192, 4096] f32, out [512, 4096] f32.

Strategy (8 NeuronCores, SPMD, f16 on-chip):
  - weight row-sharded: core d gets rows [1024d, 1024d+1024), converted to
    f16 on the host (f16 keeps 11 mantissa bits; validated end-to-end
    rel err ~1.7e-3 vs the f32 reference, gate is 2e-2). Halves HBM traffic.
  - per 128-row tile: top-8-of-each-512-chunk candidates (DVE max8),
    top-64 sorted via max8/match_replace rounds (DVE), exact sort-based
    entmax threshold recursion (Peters et al. 2019) on GPSIMD in f32,
    sqrt on Act, reconstruction r = relu(w - c) on Act, p = r*r on DVE.
    (p is 4x the true entmax output; the 1/4 is folded into x host-side.)
  - matmul x_shard @ p accumulated in PSUM over the 8 row-tiles with
    ROTATED accumulation chains: 8 psum groups in flight, group g starts
    its contraction at tile g, so the PE does useful work while later
    tiles still load. Output produced in 4 column waves of 1024 cols.
  - per wave: evacuate psum -> f16 partial [512, 1024] in DRAM ->
    ReduceScatter(add) -> this core's 64 output rows -> f32 out columns.
    The 4 ReduceScatters pipeline behind the remaining matmul waves.
"""

import numpy as np

B, IN, OUT = 512, 8192, 4096
NCORES = 8
ROWS = IN // NCORES          # 1024 weight rows per core
NT = ROWS // 128             # 8 weight tiles of [128, 4096] per core
T = 64                       # top-k length for the exact mini-entmax
NEG_FILL = -60000.0          # f16-safe "minus infinity" for match_replace
NWAVE = 4                    # column waves (RS chunks) of 1024 cols each
NB = B // 128                # 4 batch blocks

_cache = {}


COLL = "a2a"   # "rs" | "a2a"


def _build_program(variant="full"):
    from concourse import bacc, mybir, tile
    from concourse.alu_op_type import AluOpType

    f32 = mybir.dt.float32
    f16 = mybir.dt.float16

    nc = bacc.Bacc(
        "TRN2",
        target_bir_lowering=False,
        debug=False,
        enable_asserts=False,
        num_devices=NCORES,
    )

    w_ext = nc.dram_tensor("w", [ROWS, OUT], f16, kind="ExternalInput")
    # host pre-tiles xT so SBUF layout [128, (t, b)] loads contiguously:
    # xT[p, t*512 + b] = x[b, 1024d + 128t + p] * 0.25
    xT_ext = nc.dram_tensor("xT", [128, NT * B], f16, kind="ExternalInput")
    consts_ext = nc.dram_tensor("consts", [128, 2 * T], f32, kind="ExternalInput")
    out_ext = nc.dram_tensor("out", [B // NCORES, OUT], f32, kind="ExternalOutput")

    rg = [list(range(NCORES))]

    with tile.TileContext(nc) as tc:
        with (
            tc.tile_pool(name="consts", bufs=1) as cpool,
            tc.tile_pool(name="wpool", bufs=3) as wpool,
            tc.tile_pool(name="ppool", bufs=NT) as ppool,
            tc.tile_pool(name="xpool", bufs=1) as xpool,
            tc.tile_pool(name="small", bufs=2) as spool,
            tc.tile_pool(name="psum", bufs=8, space="PSUM") as psum_pool,
            tc.tile_pool(name="evac", bufs=4) as epool,
            tc.tile_pool(name="rb", bufs=2) as rpool,
            tc.tile_pool(name="dram", bufs=1, space="DRAM") as dpool,
        ):
            # ---- constants (host-provided: [:, :T]=1/rho, [:, T:]=0) ----
            cst = cpool.tile([128, 2 * T], f32, name="cst")
            nc.scalar.dma_start(out=cst[:], in_=consts_ext.ap())
            rinv = cst[:, 0:T]
            zero64 = cst[:, T:2 * T]

            # ---- per-tile entmax -> p (f16), software-pipelined ----
            # front(t): no cross-engine backpressure (cand/sort/scans/sqrt)
            # back(t):  tau-dependent tail (cond..cneg, relu, square)
            # Emitted front(t) ; back(t-1) so each engine queue always has
            # runnable work (no head-of-line blocking on the tau round-trip).
            p_tiles = []
            stash = {}

            def front(t):
                wt = wpool.tile([128, OUT], f16, name=f"wt{t}", tag="wt", bufs=3)
                nc.sync.dma_start(out=wt[:], in_=w_ext.ap()[128 * t:128 * (t + 1), :])
                if t == 0:
                    xT_sb = xpool.tile([128, NT * B], f16, name="xT_sb")
                    nc.sync.dma_start(out=xT_sb[:], in_=xT_ext.ap())
                    stash["xT"] = xT_sb

                cand = spool.tile([128, T], f16, tag="cand", bufs=2)
                for c in range(8):
                    nc.vector.max(cand[:, 8 * c:8 * c + 8], wt[:, 512 * c:512 * (c + 1)])
                v64 = spool.tile([128, T], f16, tag="v64", bufs=2)
                for j in range(8):
                    nc.vector.max(v64[:, 8 * j:8 * j + 8], cand[:])
                    if j < 7:
                        nc.vector.match_replace(
                            cand[:], v64[:, 8 * j:8 * j + 8], cand[:], NEG_FILL
                        )
                m32 = spool.tile([128, 1], f32, tag="m32", bufs=2)
                nc.gpsimd.tensor_copy(m32[:], v64[:, 0:1])
                zs = spool.tile([128, T], f32, tag="zs", bufs=2)
                nc.vector.tensor_scalar(
                    zs[:], v64[:], m32[:], 0.5, AluOpType.subtract, AluOpType.mult
                )
                zsq = spool.tile([128, T], f32, tag="zsq", bufs=2)
                nc.gpsimd.tensor_tensor(zsq[:], zs[:], zs[:], AluOpType.mult)
                cs1 = spool.tile([128, T], f32, tag="cs1", bufs=2)
                nc.vector.tensor_tensor_scan(
                    cs1[:], zs[:], zero64, 0.0, AluOpType.add, AluOpType.add
                )
                cs2 = spool.tile([128, T], f32, tag="cs2", bufs=2)
                nc.vector.tensor_tensor_scan(
                    cs2[:], zsq[:], zero64, 0.0, AluOpType.add, AluOpType.add
                )
                mean = spool.tile([128, T], f32, tag="mean", bufs=2)
                nc.gpsimd.tensor_tensor(mean[:], cs1[:], rinv, AluOpType.mult)
                msq = spool.tile([128, T], f32, tag="msq", bufs=2)
                nc.gpsimd.tensor_tensor(msq[:], cs2[:], rinv, AluOpType.mult)
                ms2 = spool.tile([128, T], f32, tag="ms2", bufs=2)
                nc.gpsimd.tensor_tensor(ms2[:], mean[:], mean[:], AluOpType.mult)
                dta = spool.tile([128, T], f32, tag="dta", bufs=2)
                nc.gpsimd.tensor_tensor(dta[:], rinv, msq[:], AluOpType.subtract)
                nc.gpsimd.tensor_tensor(dta[:], dta[:], ms2[:], AluOpType.add)
                nc.gpsimd.tensor_single_scalar(dta[:], dta[:], 0.0, AluOpType.max)
                sq = spool.tile([128, T], f32, tag="sq", bufs=2)
                nc.scalar.activation(sq[:], dta[:], mybir.ActivationFunctionType.Sqrt)
                stash[t] = (wt, m32, zs, mean, sq)

            def back(t):
                wt, m32, zs, mean, sq = stash.pop(t)
                tau = spool.tile([128, T], f32, tag="tau", bufs=2)
                nc.gpsimd.tensor_tensor(tau[:], mean[:], sq[:], AluOpType.subtract)
                # tau* = max over valid j (tau_j <= zs_j); +100 shift masks zeros
                cond = spool.tile([128, T], f32, tag="cond", bufs=2)
                nc.vector.tensor_tensor(cond[:], tau[:], zs[:], AluOpType.is_le)
                tsel = spool.tile([128, T], f32, tag="tsel", bufs=2)
                nc.vector.scalar_tensor_tensor(
                    tsel[:], tau[:], 100.0, cond[:], AluOpType.add, AluOpType.mult
                )
                tmax = spool.tile([128, 1], f32, tag="tmax", bufs=2)
                nc.vector.tensor_reduce(
                    tmax[:], tsel[:], mybir.AxisListType.X, AluOpType.max
                )
                # cneg = -(m + 2*tau*) = (200 - 2*tmax) - m
                c1 = spool.tile([128, 1], f32, tag="c1", bufs=2)
                nc.vector.tensor_scalar(
                    c1[:], tmax[:], -2.0, 200.0, AluOpType.mult, AluOpType.add
                )
                cneg = spool.tile([128, 1], f32, tag="cneg", bufs=2)
                nc.gpsimd.tensor_tensor(cneg[:], c1[:], m32[:], AluOpType.subtract)
                r = spool.tile([128, OUT], f16, tag="r", bufs=2, name=f"r{t}")
                nc.scalar.activation(
                    r[:], wt[:], mybir.ActivationFunctionType.Relu,
                    bias=cneg[:], scale=1.0,
                )
                p = ppool.tile([128, OUT], f16, tag="p", name=f"p{t}")
                nc.scalar.activation(
                    p[:], r[:], mybir.ActivationFunctionType.Square
                )
                p_tiles.append(p)

            for t in range(NT):
                front(t)
                if t >= 1:
                    back(t - 1)
            back(NT - 1)
            xT_sb = stash.pop("xT")

            # ---- matmul with rotated accumulation chains ----
            # Wave A (during loads): 8 psum groups = kq{0,1} x b{0..3}, group g
            # starts its 8-step contraction chain at tile i0=g, so the PE works
            # while tiles load. Then waves B(kq2,3), C(kq4,5), D(kq6,7).
            # Collective chunks: c0=kq0 (512 cols), c1=kq1 (512), c2=kq2,3
            # (1024), c3=kq4..7 (2048) -> first RS launches right after p_7.
            def mm_wave(kqs, rot=False):
                groups = {}
                for gi, (kq, b) in enumerate([(kq, b) for kq in kqs for b in range(NB)]):
                    ps = psum_pool.tile([128, 512], f32, tag="ps", name=f"ps{kq}_{b}")
                    groups[(kq, b)] = ps
                keys = list(groups.keys())

                def mm(gi, i, start, stop):
                    kq, b = keys[gi]
                    nc.tensor.matmul(
                        groups[(kq, b)][:],
                        lhsT=xT_sb[:, 512 * i + 128 * b:512 * i + 128 * (b + 1)],
                        rhs=p_tiles[i][:, 512 * kq:512 * (kq + 1)],
                        start=start,
                        stop=stop,
                    )

                if rot:
                    # group gi's chain = tiles gi, gi+1, .., 7, 0, .., gi-1.
                    # Emit in tile-arrival order so the in-order PE queue
                    # never blocks on a not-yet-loaded tile.
                    for j in range(NT):
                        for gi in range(j + 1):
                            mm(gi, j, start=(gi == j), stop=(gi == 0 and j == NT - 1))
                    for gi in range(1, NT):
                        for j in range(gi):
                            mm(gi, j, start=False, stop=(j == gi - 1))
                else:
                    for s in range(NT):
                        for gi in range(len(keys)):
                            mm(gi, s, start=(s == 0), stop=(s == NT - 1))
                return groups

            def evac_dma(groups, kqs, partial, col0, engine_flip=0):
                # copy psum -> f16 and DMA into partial[:, colrange]
                for b in range(NB):
                    ncols = 512 * len(kqs)
                    ev = epool.tile([128, ncols], f16, tag="ev", bufs=4,
                                    name=f"ev{kqs[0]}_{b}")
                    for k, kq in enumerate(kqs):
                        if (b + k + engine_flip) % 2 == 0:
                            nc.vector.tensor_copy(
                                ev[:, 512 * k:512 * (k + 1)], groups[(kq, b)][:]
                            )
                        else:
                            nc.scalar.copy(
                                ev[:, 512 * k:512 * (k + 1)], groups[(kq, b)][:]
                            )
                    nc.sync.dma_start(
                        out=partial[128 * b:128 * (b + 1),
                                    col0:col0 + ncols],
                        in_=ev[:],
                    )

            finishes = []

            def rs_chunk(partial, ncols, ocol0, w):
                rsout = dpool.tile([B // NCORES, ncols], f16, name=f"rsout{w}")
                nc.gpsimd.collective_compute(
                    "ReduceScatter",
                    mybir.AluOpType.add,
                    replica_groups=rg,
                    ins=[partial.opt()],
                    outs=[rsout.opt()],
                )
                finishes.append((rsout, ncols, ocol0, w))

            def rs_finish():
                # deferred so rb waits never block evac/dma queues of later waves
                for rsout, ncols, ocol0, w in finishes:
                    rb = rpool.tile([64, ncols], f16, tag=f"rb{w}", bufs=1)
                    nc.gpsimd.dma_start(out=rb[:], in_=rsout[:])
                    rb32 = rpool.tile([64, ncols], f32, tag=f"rb32{w}", bufs=1)
                    h = ncols // 2
                    nc.vector.tensor_copy(rb32[:, 0:h], rb[:, 0:h])
                    nc.scalar.copy(rb32[:, h:ncols], rb[:, h:ncols])
                    nc.sync.dma_start(
                        out=out_ext.ap()[:, ocol0:ocol0 + ncols], in_=rb32[:]
                    )

            # Wave A: kq 0 and 1 concurrently (8 psum banks), rotated chains
            gA = mm_wave([0, 1], rot=True)
            pc0 = dpool.tile([B, 1024], f16, name="partial0")
            evac_dma(gA, [0, 1], pc0, 0)
            rs_chunk(pc0, 1024, 0, 0)

            # Wave B: kq 2,3
            gB = mm_wave([2, 3])
            pc1 = dpool.tile([B, 1024], f16, name="partial1")
            evac_dma(gB, [2, 3], pc1, 0, engine_flip=1)
            rs_chunk(pc1, 1024, 1024, 1)

            # Waves C, D: kq 4..7 -> one 2048-col chunk
            pc2 = dpool.tile([B, 2048], f16, name="partial2")
            gC = mm_wave([4, 5])
            evac_dma(gC, [4, 5], pc2, 0)
            gD = mm_wave([6, 7])
            evac_dma(gD, [6, 7], pc2, 1024, engine_flip=1)
            rs_chunk(pc2, 2048, 2048, 2)
            rs_finish()

    nc.compile()
    return nc


def get_program():
    if "nc" not in _cache:
        _cache["nc"] = _build_program()
    return _cache["nc"]


def kernel(x: np.ndarray, weight: np.ndarray, trace: bool = False):
    from concourse.bass_utils import run_bass_kernel_spmd

    x = np.ascontiguousarray(x, dtype=np.float32)
    weight = np.ascontiguousarray(weight, dtype=np.float32)
    assert x.shape == (B, IN) and weight.shape == (IN, OUT)

    nc = get_program()
    in_maps = []
    for d in range(NCORES):
        wsh = np.ascontiguousarray(
            weight[ROWS * d:ROWS * (d + 1), :], dtype=np.float16
        )
        # xT[p, t*512 + b] = 0.25 * x[b, 1024d + 128t + p]
        xsh = (0.25 * x[:, ROWS * d:ROWS * (d + 1)].T).astype(np.float16)
        xt = np.ascontiguousarray(
            xsh.reshape(NT, 128, B).transpose(1, 0, 2).reshape(128, NT * B)
        )
        rho = np.arange(1, T + 1, dtype=np.float32)
        cst = np.zeros((128, 2 * T), dtype=np.float32)
        cst[:, 0:T] = 1.0 / rho
        in_maps.append({"w": wsh, "xT": xt, "consts": cst})
    res = run_bass_kernel_spmd(
        nc, in_maps, core_ids=list(range(NCORES)), trace=trace
    )
    out = np.concatenate(
        [res.results[d]["out"] for d in range(NCORES)], axis=0
    )
    if trace:
        _cache["last_result"] = res
    return out


# revision 20
# speedup vs baseline: 1.0299x; 1.0299x over previous
"""EntMaxSelectLayer distributed Trainium2 kernel (v2).

Computes out = x @ entmax15(weight, axis=-1) with
  x [512, 8192] f32, weight [8192, 4096] f32, out [512, 4096] f32.

Strategy (8 NeuronCores, SPMD, f16 on-chip):
  - weight row-sharded: core d gets rows [1024d, 1024d+1024), converted to
    f16 on the host (f16 keeps 11 mantissa bits; validated end-to-end
    rel err ~1.7e-3 vs the f32 reference, gate is 2e-2). Halves HBM traffic.
  - per 128-row tile: top-8-of-each-512-chunk candidates (DVE max8),
    top-64 sorted via max8/match_replace rounds (DVE), exact sort-based
    entmax threshold recursion (Peters et al. 2019) on GPSIMD in f32,
    sqrt on Act, reconstruction r = relu(w - c) on Act, p = r*r on DVE.
    (p is 4x the true entmax output; the 1/4 is folded into x host-side.)
  - matmul x_shard @ p accumulated in PSUM over the 8 row-tiles with
    ROTATED accumulation chains: 8 psum groups in flight, group g starts
    its contraction at tile g, so the PE does useful work while later
    tiles still load. Output produced in 4 column waves of 1024 cols.
  - per wave: evacuate psum -> f16 partial [512, 1024] in DRAM ->
    ReduceScatter(add) -> this core's 64 output rows -> f32 out columns.
    The 4 ReduceScatters pipeline behind the remaining matmul waves.
"""

import numpy as np

B, IN, OUT = 512, 8192, 4096
NCORES = 8
ROWS = IN // NCORES          # 1024 weight rows per core
NT = ROWS // 128             # 8 weight tiles of [128, 4096] per core
T = 64                       # top-k length for the exact mini-entmax
NEG_FILL = -60000.0          # f16-safe "minus infinity" for match_replace
NWAVE = 4                    # column waves (RS chunks) of 1024 cols each
NB = B // 128                # 4 batch blocks

_cache = {}


COLL = "a2a"   # "rs" | "a2a"


def _build_program(variant="full"):
    from concourse import bacc, mybir, tile
    from concourse.alu_op_type import AluOpType

    f32 = mybir.dt.float32
    f16 = mybir.dt.float16

    nc = bacc.Bacc(
        "TRN2",
        target_bir_lowering=False,
        debug=False,
        enable_asserts=False,
        num_devices=NCORES,
    )

    w_ext = nc.dram_tensor("w", [ROWS, OUT], f16, kind="ExternalInput")
    # host pre-tiles xT so SBUF layout [128, (t, b)] loads contiguously:
    # xT[p, t*512 + b] = x[b, 1024d + 128t + p] * 0.25
    xT_ext = nc.dram_tensor("xT", [128, NT * B], f16, kind="ExternalInput")
    consts_ext = nc.dram_tensor("consts", [128, 2 * T], f32, kind="ExternalInput")
    out_ext = nc.dram_tensor("out", [B // NCORES, OUT], f32, kind="ExternalOutput")

    rg = [list(range(NCORES))]

    with tile.TileContext(nc) as tc:
        with (
            tc.tile_pool(name="consts", bufs=1) as cpool,
            tc.tile_pool(name="wpool", bufs=3) as wpool,
            tc.tile_pool(name="ppool", bufs=NT) as ppool,
            tc.tile_pool(name="xpool", bufs=1) as xpool,
            tc.tile_pool(name="small", bufs=2) as spool,
            tc.tile_pool(name="psum", bufs=8, space="PSUM") as psum_pool,
            tc.tile_pool(name="evac", bufs=4) as epool,
            tc.tile_pool(name="rb", bufs=2) as rpool,
            tc.tile_pool(name="dram", bufs=1, space="DRAM") as dpool,
        ):
            # ---- constants (host-provided: [:, :T]=1/rho, [:, T:]=0) ----
            cst = cpool.tile([128, 2 * T], f32, name="cst")
            nc.scalar.dma_start(out=cst[:], in_=consts_ext.ap())

            # tiny barrier collective up-front: absorbs NRT startup stagger
            # while the cores are otherwise idle, so the real ReduceScatters
            # later do not each pay the rank-skew wait.
            bar_in = dpool.tile([1, 64], f16, name="bar_in")
            bar_out = dpool.tile([8, 64], f16, name="bar_out")
            nc.gpsimd.collective_compute(
                "AllGather",
                mybir.AluOpType.bypass,
                replica_groups=rg,
                ins=[bar_in.opt()],
                outs=[bar_out.opt()],
            )
            rinv = cst[:, 0:T]
            zero64 = cst[:, T:2 * T]

            # ---- per-tile entmax -> p (f16), software-pipelined ----
            # front(t): no cross-engine backpressure (cand/sort/scans/sqrt)
            # back(t):  tau-dependent tail (cond..cneg, relu, square)
            # Emitted front(t) ; back(t-1) so each engine queue always has
            # runnable work (no head-of-line blocking on the tau round-trip).
            p_tiles = []
            stash = {}

            def front(t):
                wt = wpool.tile([128, OUT], f16, name=f"wt{t}", tag="wt", bufs=3)
                nc.sync.dma_start(out=wt[:], in_=w_ext.ap()[128 * t:128 * (t + 1), :])
                if t == 0:
                    xT_sb = xpool.tile([128, NT * B], f16, name="xT_sb")
                    nc.sync.dma_start(out=xT_sb[:], in_=xT_ext.ap())
                    stash["xT"] = xT_sb

                cand = spool.tile([128, T], f16, tag="cand", bufs=2)
                for c in range(8):
                    nc.vector.max(cand[:, 8 * c:8 * c + 8], wt[:, 512 * c:512 * (c + 1)])
                v64 = spool.tile([128, T], f16, tag="v64", bufs=2)
                for j in range(8):
                    nc.vector.max(v64[:, 8 * j:8 * j + 8], cand[:])
                    if j < 7:
                        nc.vector.match_replace(
                            cand[:], v64[:, 8 * j:8 * j + 8], cand[:], NEG_FILL
                        )
                m32 = spool.tile([128, 1], f32, tag="m32", bufs=2)
                nc.gpsimd.tensor_copy(m32[:], v64[:, 0:1])
                zs = spool.tile([128, T], f32, tag="zs", bufs=2)
                nc.vector.tensor_scalar(
                    zs[:], v64[:], m32[:], 0.5, AluOpType.subtract, AluOpType.mult
                )
                zsq = spool.tile([128, T], f32, tag="zsq", bufs=2)
                nc.gpsimd.tensor_tensor(zsq[:], zs[:], zs[:], AluOpType.mult)
                cs1 = spool.tile([128, T], f32, tag="cs1", bufs=2)
                nc.vector.tensor_tensor_scan(
                    cs1[:], zs[:], zero64, 0.0, AluOpType.add, AluOpType.add
                )
                cs2 = spool.tile([128, T], f32, tag="cs2", bufs=2)
                nc.vector.tensor_tensor_scan(
                    cs2[:], zsq[:], zero64, 0.0, AluOpType.add, AluOpType.add
                )
                mean = spool.tile([128, T], f32, tag="mean", bufs=2)
                nc.gpsimd.tensor_tensor(mean[:], cs1[:], rinv, AluOpType.mult)
                msq = spool.tile([128, T], f32, tag="msq", bufs=2)
                nc.gpsimd.tensor_tensor(msq[:], cs2[:], rinv, AluOpType.mult)
                ms2 = spool.tile([128, T], f32, tag="ms2", bufs=2)
                nc.gpsimd.tensor_tensor(ms2[:], mean[:], mean[:], AluOpType.mult)
                dta = spool.tile([128, T], f32, tag="dta", bufs=2)
                nc.gpsimd.tensor_tensor(dta[:], rinv, msq[:], AluOpType.subtract)
                nc.gpsimd.tensor_tensor(dta[:], dta[:], ms2[:], AluOpType.add)
                nc.gpsimd.tensor_single_scalar(dta[:], dta[:], 0.0, AluOpType.max)
                sq = spool.tile([128, T], f32, tag="sq", bufs=2)
                nc.scalar.activation(sq[:], dta[:], mybir.ActivationFunctionType.Sqrt)
                stash[t] = (wt, m32, zs, mean, sq)

            def back(t):
                wt, m32, zs, mean, sq = stash.pop(t)
                tau = spool.tile([128, T], f32, tag="tau", bufs=2)
                nc.gpsimd.tensor_tensor(tau[:], mean[:], sq[:], AluOpType.subtract)
                # tau* = max over valid j (tau_j <= zs_j); +100 shift masks zeros
                cond = spool.tile([128, T], f32, tag="cond", bufs=2)
                nc.vector.tensor_tensor(cond[:], tau[:], zs[:], AluOpType.is_le)
                tsel = spool.tile([128, T], f32, tag="tsel", bufs=2)
                nc.vector.scalar_tensor_tensor(
                    tsel[:], tau[:], 100.0, cond[:], AluOpType.add, AluOpType.mult
                )
                tmax = spool.tile([128, 1], f32, tag="tmax", bufs=2)
                nc.vector.tensor_reduce(
                    tmax[:], tsel[:], mybir.AxisListType.X, AluOpType.max
                )
                # cneg = -(m + 2*tau*) = (200 - 2*tmax) - m
                c1 = spool.tile([128, 1], f32, tag="c1", bufs=2)
                nc.vector.tensor_scalar(
                    c1[:], tmax[:], -2.0, 200.0, AluOpType.mult, AluOpType.add
                )
                cneg = spool.tile([128, 1], f32, tag="cneg", bufs=2)
                nc.gpsimd.tensor_tensor(cneg[:], c1[:], m32[:], AluOpType.subtract)
                r = spool.tile([128, OUT], f16, tag="r", bufs=2, name=f"r{t}")
                nc.scalar.activation(
                    r[:], wt[:], mybir.ActivationFunctionType.Relu,
                    bias=cneg[:], scale=1.0,
                )
                p = ppool.tile([128, OUT], f16, tag="p", name=f"p{t}")
                nc.scalar.activation(
                    p[:], r[:], mybir.ActivationFunctionType.Square
                )
                p_tiles.append(p)

            for t in range(NT):
                front(t)
                if t >= 1:
                    back(t - 1)
            back(NT - 1)
            xT_sb = stash.pop("xT")

            # ---- matmul with rotated accumulation chains ----
            # Wave A (during loads): 8 psum groups = kq{0,1} x b{0..3}, group g
            # starts its 8-step contraction chain at tile i0=g, so the PE works
            # while tiles load. Then waves B(kq2,3), C(kq4,5), D(kq6,7).
            # Collective chunks: c0=kq0 (512 cols), c1=kq1 (512), c2=kq2,3
            # (1024), c3=kq4..7 (2048) -> first RS launches right after p_7.
            def mm_wave(kqs, rot=False):
                groups = {}
                for gi, (kq, b) in enumerate([(kq, b) for kq in kqs for b in range(NB)]):
                    ps = psum_pool.tile([128, 512], f32, tag="ps", name=f"ps{kq}_{b}")
                    groups[(kq, b)] = ps
                keys = list(groups.keys())

                def mm(gi, i, start, stop):
                    kq, b = keys[gi]
                    nc.tensor.matmul(
                        groups[(kq, b)][:],
                        lhsT=xT_sb[:, 512 * i + 128 * b:512 * i + 128 * (b + 1)],
                        rhs=p_tiles[i][:, 512 * kq:512 * (kq + 1)],
                        start=start,
                        stop=stop,
                    )

                if rot:
                    # group gi's chain = tiles gi, gi+1, .., 7, 0, .., gi-1.
                    # Emit in tile-arrival order so the in-order PE queue
                    # never blocks on a not-yet-loaded tile.
                    for j in range(NT):
                        for gi in range(j + 1):
                            mm(gi, j, start=(gi == j), stop=(gi == 0 and j == NT - 1))
                    for gi in range(1, NT):
                        for j in range(gi):
                            mm(gi, j, start=False, stop=(j == gi - 1))
                else:
                    for s in range(NT):
                        for gi in range(len(keys)):
                            mm(gi, s, start=(s == 0), stop=(s == NT - 1))
                return groups

            def evac_dma(groups, kqs, partial, col0, engine_flip=0):
                # copy psum -> f16 and DMA into partial[:, colrange]
                for b in range(NB):
                    ncols = 512 * len(kqs)
                    ev = epool.tile([128, ncols], f16, tag="ev", bufs=4,
                                    name=f"ev{kqs[0]}_{b}")
                    for k, kq in enumerate(kqs):
                        if (b + k + engine_flip) % 2 == 0:
                            nc.vector.tensor_copy(
                                ev[:, 512 * k:512 * (k + 1)], groups[(kq, b)][:]
                            )
                        else:
                            nc.scalar.copy(
                                ev[:, 512 * k:512 * (k + 1)], groups[(kq, b)][:]
                            )
                    nc.sync.dma_start(
                        out=partial[128 * b:128 * (b + 1),
                                    col0:col0 + ncols],
                        in_=ev[:],
                    )

            finishes = []

            def rs_chunk(partial, ncols, ocol0, w):
                rsout = dpool.tile([B // NCORES, ncols], f16, name=f"rsout{w}")
                nc.gpsimd.collective_compute(
                    "ReduceScatter",
                    mybir.AluOpType.add,
                    replica_groups=rg,
                    ins=[partial.opt()],
                    outs=[rsout.opt()],
                )
                finishes.append((rsout, ncols, ocol0, w))

            def rs_finish():
                # deferred so rb waits never block evac/dma queues of later waves
                for rsout, ncols, ocol0, w in finishes:
                    rb = rpool.tile([64, ncols], f16, tag=f"rb{w}", bufs=1)
                    nc.gpsimd.dma_start(out=rb[:], in_=rsout[:])
                    rb32 = rpool.tile([64, ncols], f32, tag=f"rb32{w}", bufs=1)
                    h = ncols // 2
                    nc.vector.tensor_copy(rb32[:, 0:h], rb[:, 0:h])
                    nc.scalar.copy(rb32[:, h:ncols], rb[:, h:ncols])
                    nc.sync.dma_start(
                        out=out_ext.ap()[:, ocol0:ocol0 + ncols], in_=rb32[:]
                    )

            # Wave A: kq 0 and 1 concurrently (8 psum banks), rotated chains
            gA = mm_wave([0, 1], rot=True)
            pc0 = dpool.tile([B, 1024], f16, name="partial0")
            evac_dma(gA, [0, 1], pc0, 0)
            rs_chunk(pc0, 1024, 0, 0)

            # Wave B: kq 2,3
            gB = mm_wave([2, 3])
            pc1 = dpool.tile([B, 1024], f16, name="partial1")
            evac_dma(gB, [2, 3], pc1, 0, engine_flip=1)
            rs_chunk(pc1, 1024, 1024, 1)

            # Waves C, D: kq 4..7 -> one 2048-col chunk
            pc2 = dpool.tile([B, 2048], f16, name="partial2")
            gC = mm_wave([4, 5])
            evac_dma(gC, [4, 5], pc2, 0)
            gD = mm_wave([6, 7])
            evac_dma(gD, [6, 7], pc2, 1024, engine_flip=1)
            rs_chunk(pc2, 2048, 2048, 2)
            rs_finish()

    nc.compile()
    return nc


def get_program():
    if "nc" not in _cache:
        _cache["nc"] = _build_program()
    return _cache["nc"]


def kernel(x: np.ndarray, weight: np.ndarray, trace: bool = False):
    from concourse.bass_utils import run_bass_kernel_spmd

    x = np.ascontiguousarray(x, dtype=np.float32)
    weight = np.ascontiguousarray(weight, dtype=np.float32)
    assert x.shape == (B, IN) and weight.shape == (IN, OUT)

    nc = get_program()
    in_maps = []
    for d in range(NCORES):
        wsh = np.ascontiguousarray(
            weight[ROWS * d:ROWS * (d + 1), :], dtype=np.float16
        )
        # xT[p, t*512 + b] = 0.25 * x[b, 1024d + 128t + p]
        xsh = (0.25 * x[:, ROWS * d:ROWS * (d + 1)].T).astype(np.float16)
        xt = np.ascontiguousarray(
            xsh.reshape(NT, 128, B).transpose(1, 0, 2).reshape(128, NT * B)
        )
        rho = np.arange(1, T + 1, dtype=np.float32)
        cst = np.zeros((128, 2 * T), dtype=np.float32)
        cst[:, 0:T] = 1.0 / rho
        in_maps.append({"w": wsh, "xT": xt, "consts": cst})
    res = run_bass_kernel_spmd(
        nc, in_maps, core_ids=list(range(NCORES)), trace=trace
    )
    out = np.concatenate(
        [res.results[d]["out"] for d in range(NCORES)], axis=0
    )
    if trace:
        _cache["last_result"] = res
    return out


# revision 21
# speedup vs baseline: 1.0869x; 1.0553x over previous
"""EntMaxSelectLayer distributed Trainium2 kernel (v2).

Computes out = x @ entmax15(weight, axis=-1) with
  x [512, 8192] f32, weight [8192, 4096] f32, out [512, 4096] f32.

Strategy (8 NeuronCores, SPMD, f16 on-chip):
  - weight row-sharded: core d gets rows [1024d, 1024d+1024), converted to
    f16 on the host (f16 keeps 11 mantissa bits; validated end-to-end
    rel err ~1.7e-3 vs the f32 reference, gate is 2e-2). Halves HBM traffic.
  - per 128-row tile: top-8-of-each-512-chunk candidates (DVE max8),
    top-64 sorted via max8/match_replace rounds (DVE), exact sort-based
    entmax threshold recursion (Peters et al. 2019) on GPSIMD in f32,
    sqrt on Act, reconstruction r = relu(w - c) on Act, p = r*r on DVE.
    (p is 4x the true entmax output; the 1/4 is folded into x host-side.)
  - matmul x_shard @ p accumulated in PSUM over the 8 row-tiles with
    ROTATED accumulation chains: 8 psum groups in flight, group g starts
    its contraction at tile g, so the PE does useful work while later
    tiles still load. Output produced in 4 column waves of 1024 cols.
  - per wave: evacuate psum -> f16 partial [512, 1024] in DRAM ->
    ReduceScatter(add) -> this core's 64 output rows -> f32 out columns.
    The 4 ReduceScatters pipeline behind the remaining matmul waves.
"""

import numpy as np

B, IN, OUT = 512, 8192, 4096
NCORES = 8
ROWS = IN // NCORES          # 1024 weight rows per core
NT = ROWS // 128             # 8 weight tiles of [128, 4096] per core
T = 56                       # sorted prefix for the exact mini-entmax
NCAND = 64                   # candidate count (top-8 of each 512-chunk)
NEG_FILL = -60000.0          # f16-safe "minus infinity" for match_replace
NWAVE = 4                    # column waves (RS chunks) of 1024 cols each
NB = B // 128                # 4 batch blocks

_cache = {}


COLL = "a2a"   # "rs" | "a2a"


def _build_program(variant="full"):
    from concourse import bacc, mybir, tile
    from concourse.alu_op_type import AluOpType

    f32 = mybir.dt.float32
    f16 = mybir.dt.float16

    nc = bacc.Bacc(
        "TRN2",
        target_bir_lowering=False,
        debug=False,
        enable_asserts=False,
        num_devices=NCORES,
    )

    w_ext = nc.dram_tensor("w", [ROWS, OUT], f16, kind="ExternalInput")
    # host pre-tiles xT so SBUF layout [128, (t, b)] loads contiguously:
    # xT[p, t*512 + b] = x[b, 1024d + 128t + p] * 0.25
    xT_ext = nc.dram_tensor("xT", [128, NT * B], f16, kind="ExternalInput")
    consts_ext = nc.dram_tensor("consts", [128, 2 * T], f32, kind="ExternalInput")
    out_ext = nc.dram_tensor("out", [B // NCORES, OUT], f32, kind="ExternalOutput")

    rg = [list(range(NCORES))]

    with tile.TileContext(nc) as tc:
        with (
            tc.tile_pool(name="consts", bufs=1) as cpool,
            tc.tile_pool(name="wpool", bufs=3) as wpool,
            tc.tile_pool(name="ppool", bufs=NT) as ppool,
            tc.tile_pool(name="xpool", bufs=1) as xpool,
            tc.tile_pool(name="small", bufs=2) as spool,
            tc.tile_pool(name="psum", bufs=8, space="PSUM") as psum_pool,
            tc.tile_pool(name="evac", bufs=4) as epool,
            tc.tile_pool(name="rb", bufs=2) as rpool,
            tc.tile_pool(name="dram", bufs=1, space="DRAM") as dpool,
        ):
            # ---- constants (host-provided: [:, :T]=1/rho, [:, T:]=0) ----
            cst = cpool.tile([128, 2 * T], f32, name="cst")
            nc.scalar.dma_start(out=cst[:], in_=consts_ext.ap())

            # tiny barrier collective up-front: absorbs NRT startup stagger
            # while the cores are otherwise idle, so the real ReduceScatters
            # later do not each pay the rank-skew wait.
            bar_in = dpool.tile([1, 64], f16, name="bar_in")
            bar_out = dpool.tile([8, 64], f16, name="bar_out")
            nc.gpsimd.collective_compute(
                "AllGather",
                mybir.AluOpType.bypass,
                replica_groups=rg,
                ins=[bar_in.opt()],
                outs=[bar_out.opt()],
            )
            rinv = cst[:, 0:T]
            zero64 = cst[:, T:2 * T]

            # ---- per-tile entmax -> p (f16), software-pipelined ----
            # front(t): no cross-engine backpressure (cand/sort/scans/sqrt)
            # back(t):  tau-dependent tail (cond..cneg, relu, square)
            # Emitted front(t) ; back(t-1) so each engine queue always has
            # runnable work (no head-of-line blocking on the tau round-trip).
            p_tiles = []
            stash = {}

            def front(t):
                wt = wpool.tile([128, OUT], f16, name=f"wt{t}", tag="wt", bufs=3)
                nc.sync.dma_start(out=wt[:], in_=w_ext.ap()[128 * t:128 * (t + 1), :])
                if t == 0:
                    xT_sb = xpool.tile([128, NT * B], f16, name="xT_sb")
                    nc.sync.dma_start(out=xT_sb[:], in_=xT_ext.ap())
                    stash["xT"] = xT_sb

                cand = spool.tile([128, NCAND], f16, tag="cand", bufs=2)
                for c in range(8):
                    nc.vector.max(cand[:, 8 * c:8 * c + 8], wt[:, 512 * c:512 * (c + 1)])
                v64 = spool.tile([128, T], f16, tag="v64", bufs=2)
                for j in range(T // 8):
                    nc.vector.max(v64[:, 8 * j:8 * j + 8], cand[:])
                    if j < T // 8 - 1:
                        nc.vector.match_replace(
                            cand[:], v64[:, 8 * j:8 * j + 8], cand[:], NEG_FILL
                        )
                m32 = spool.tile([128, 1], f32, tag="m32", bufs=2)
                nc.gpsimd.tensor_copy(m32[:], v64[:, 0:1])
                zs = spool.tile([128, T], f32, tag="zs", bufs=2)
                nc.vector.tensor_scalar(
                    zs[:], v64[:], m32[:], 0.5, AluOpType.subtract, AluOpType.mult
                )
                zsq = spool.tile([128, T], f32, tag="zsq", bufs=2)
                nc.gpsimd.tensor_tensor(zsq[:], zs[:], zs[:], AluOpType.mult)
                cs1 = spool.tile([128, T], f32, tag="cs1", bufs=2)
                nc.vector.tensor_tensor_scan(
                    cs1[:], zs[:], zero64, 0.0, AluOpType.add, AluOpType.add
                )
                cs2 = spool.tile([128, T], f32, tag="cs2", bufs=2)
                nc.vector.tensor_tensor_scan(
                    cs2[:], zsq[:], zero64, 0.0, AluOpType.add, AluOpType.add
                )
                mean = spool.tile([128, T], f32, tag="mean", bufs=2)
                nc.gpsimd.tensor_tensor(mean[:], cs1[:], rinv, AluOpType.mult)
                msq = spool.tile([128, T], f32, tag="msq", bufs=2)
                nc.gpsimd.tensor_tensor(msq[:], cs2[:], rinv, AluOpType.mult)
                ms2 = spool.tile([128, T], f32, tag="ms2", bufs=2)
                nc.gpsimd.tensor_tensor(ms2[:], mean[:], mean[:], AluOpType.mult)
                dta = spool.tile([128, T], f32, tag="dta", bufs=2)
                nc.gpsimd.tensor_tensor(dta[:], rinv, msq[:], AluOpType.subtract)
                nc.gpsimd.tensor_tensor(dta[:], dta[:], ms2[:], AluOpType.add)
                nc.gpsimd.tensor_single_scalar(dta[:], dta[:], 0.0, AluOpType.max)
                sq = spool.tile([128, T], f32, tag="sq", bufs=2)
                nc.scalar.activation(sq[:], dta[:], mybir.ActivationFunctionType.Sqrt)
                stash[t] = (wt, m32, zs, mean, sq)

            def back(t):
                wt, m32, zs, mean, sq = stash.pop(t)
                tau = spool.tile([128, T], f32, tag="tau", bufs=2)
                nc.gpsimd.tensor_tensor(tau[:], mean[:], sq[:], AluOpType.subtract)
                # tau* = max over valid j (tau_j <= zs_j); +100 shift masks zeros
                cond = spool.tile([128, T], f32, tag="cond", bufs=2)
                nc.vector.tensor_tensor(cond[:], tau[:], zs[:], AluOpType.is_le)
                tsel = spool.tile([128, T], f32, tag="tsel", bufs=2)
                nc.vector.scalar_tensor_tensor(
                    tsel[:], tau[:], 100.0, cond[:], AluOpType.add, AluOpType.mult
                )
                tmax = spool.tile([128, 1], f32, tag="tmax", bufs=2)
                nc.vector.tensor_reduce(
                    tmax[:], tsel[:], mybir.AxisListType.X, AluOpType.max
                )
                # cneg = -(m + 2*tau*) = (200 - 2*tmax) - m
                c1 = spool.tile([128, 1], f32, tag="c1", bufs=2)
                nc.vector.tensor_scalar(
                    c1[:], tmax[:], -2.0, 200.0, AluOpType.mult, AluOpType.add
                )
                cneg = spool.tile([128, 1], f32, tag="cneg", bufs=2)
                nc.gpsimd.tensor_tensor(cneg[:], c1[:], m32[:], AluOpType.subtract)
                r = spool.tile([128, OUT], f16, tag="r", bufs=2, name=f"r{t}")
                nc.scalar.activation(
                    r[:], wt[:], mybir.ActivationFunctionType.Relu,
                    bias=cneg[:], scale=1.0,
                )
                p = ppool.tile([128, OUT], f16, tag="p", name=f"p{t}")
                nc.scalar.activation(
                    p[:], r[:], mybir.ActivationFunctionType.Square
                )
                p_tiles.append(p)

            for t in range(NT):
                front(t)
                if t >= 1:
                    back(t - 1)
            back(NT - 1)
            xT_sb = stash.pop("xT")

            # ---- matmul with rotated accumulation chains ----
            # Wave A (during loads): 8 psum groups = kq{0,1} x b{0..3}, group g
            # starts its 8-step contraction chain at tile i0=g, so the PE works
            # while tiles load. Then waves B(kq2,3), C(kq4,5), D(kq6,7).
            # Collective chunks: c0=kq0 (512 cols), c1=kq1 (512), c2=kq2,3
            # (1024), c3=kq4..7 (2048) -> first RS launches right after p_7.
            def mm_wave(kqs, rot=False):
                groups = {}
                for gi, (kq, b) in enumerate([(kq, b) for kq in kqs for b in range(NB)]):
                    ps = psum_pool.tile([128, 512], f32, tag="ps", name=f"ps{kq}_{b}")
                    groups[(kq, b)] = ps
                keys = list(groups.keys())

                def mm(gi, i, start, stop):
                    kq, b = keys[gi]
                    nc.tensor.matmul(
                        groups[(kq, b)][:],
                        lhsT=xT_sb[:, 512 * i + 128 * b:512 * i + 128 * (b + 1)],
                        rhs=p_tiles[i][:, 512 * kq:512 * (kq + 1)],
                        start=start,
                        stop=stop,
                    )

                if rot:
                    # group gi's chain = tiles gi, gi+1, .., 7, 0, .., gi-1.
                    # Emit in tile-arrival order so the in-order PE queue
                    # never blocks on a not-yet-loaded tile.
                    for j in range(NT):
                        for gi in range(j + 1):
                            mm(gi, j, start=(gi == j), stop=(gi == 0 and j == NT - 1))
                    for gi in range(1, NT):
                        for j in range(gi):
                            mm(gi, j, start=False, stop=(j == gi - 1))
                else:
                    for s in range(NT):
                        for gi in range(len(keys)):
                            mm(gi, s, start=(s == 0), stop=(s == NT - 1))
                return groups

            def evac_dma(groups, kqs, partial, col0, engine_flip=0):
                # copy psum -> f16 and DMA into partial[:, colrange]
                for b in range(NB):
                    ncols = 512 * len(kqs)
                    ev = epool.tile([128, ncols], f16, tag="ev", bufs=4,
                                    name=f"ev{kqs[0]}_{b}")
                    for k, kq in enumerate(kqs):
                        if (b + k + engine_flip) % 2 == 0:
                            nc.vector.tensor_copy(
                                ev[:, 512 * k:512 * (k + 1)], groups[(kq, b)][:]
                            )
                        else:
                            nc.scalar.copy(
                                ev[:, 512 * k:512 * (k + 1)], groups[(kq, b)][:]
                            )
                    nc.sync.dma_start(
                        out=partial[128 * b:128 * (b + 1),
                                    col0:col0 + ncols],
                        in_=ev[:],
                    )

            finishes = []

            def rs_chunk(partial, ncols, ocol0, w):
                rsout = dpool.tile([B // NCORES, ncols], f16, name=f"rsout{w}")
                nc.gpsimd.collective_compute(
                    "ReduceScatter",
                    mybir.AluOpType.add,
                    replica_groups=rg,
                    ins=[partial.opt()],
                    outs=[rsout.opt()],
                )
                finishes.append((rsout, ncols, ocol0, w))

            def rs_finish():
                # deferred so rb waits never block evac/dma queues of later waves
                for rsout, ncols, ocol0, w in finishes:
                    rb = rpool.tile([64, ncols], f16, tag=f"rb{w}", bufs=1)
                    nc.gpsimd.dma_start(out=rb[:], in_=rsout[:])
                    rb32 = rpool.tile([64, ncols], f32, tag=f"rb32{w}", bufs=1)
                    h = ncols // 2
                    nc.vector.tensor_copy(rb32[:, 0:h], rb[:, 0:h])
                    nc.scalar.copy(rb32[:, h:ncols], rb[:, h:ncols])
                    nc.sync.dma_start(
                        out=out_ext.ap()[:, ocol0:ocol0 + ncols], in_=rb32[:]
                    )

            # Wave A: kq 0 and 1 concurrently (8 psum banks), rotated chains
            gA = mm_wave([0, 1], rot=True)
            pc0 = dpool.tile([B, 1024], f16, name="partial0")
            evac_dma(gA, [0, 1], pc0, 0)
            rs_chunk(pc0, 1024, 0, 0)

            # Wave B: kq 2,3
            gB = mm_wave([2, 3])
            pc1 = dpool.tile([B, 1024], f16, name="partial1")
            evac_dma(gB, [2, 3], pc1, 0, engine_flip=1)
            rs_chunk(pc1, 1024, 1024, 1)

            # Waves C, D: kq 4..7 -> one 2048-col chunk
            pc2 = dpool.tile([B, 2048], f16, name="partial2")
            gC = mm_wave([4, 5])
            evac_dma(gC, [4, 5], pc2, 0)
            gD = mm_wave([6, 7])
            evac_dma(gD, [6, 7], pc2, 1024, engine_flip=1)
            rs_chunk(pc2, 2048, 2048, 2)
            rs_finish()

    nc.compile()
    return nc


def get_program():
    if "nc" not in _cache:
        _cache["nc"] = _build_program()
    return _cache["nc"]


def kernel(x: np.ndarray, weight: np.ndarray, trace: bool = False):
    from concourse.bass_utils import run_bass_kernel_spmd

    x = np.ascontiguousarray(x, dtype=np.float32)
    weight = np.ascontiguousarray(weight, dtype=np.float32)
    assert x.shape == (B, IN) and weight.shape == (IN, OUT)

    nc = get_program()
    in_maps = []
    for d in range(NCORES):
        wsh = np.ascontiguousarray(
            weight[ROWS * d:ROWS * (d + 1), :], dtype=np.float16
        )
        # xT[p, t*512 + b] = 0.25 * x[b, 1024d + 128t + p]
        xsh = (0.25 * x[:, ROWS * d:ROWS * (d + 1)].T).astype(np.float16)
        xt = np.ascontiguousarray(
            xsh.reshape(NT, 128, B).transpose(1, 0, 2).reshape(128, NT * B)
        )
        rho = np.arange(1, T + 1, dtype=np.float32)
        cst = np.zeros((128, 2 * T), dtype=np.float32)
        cst[:, 0:T] = 1.0 / rho
        in_maps.append({"w": wsh, "xT": xt, "consts": cst})
    res = run_bass_kernel_spmd(
        nc, in_maps, core_ids=list(range(NCORES)), trace=trace
    )
    out = np.concatenate(
        [res.results[d]["out"] for d in range(NCORES)], axis=0
    )
    if trace:
        _cache["last_result"] = res
    return out
